# revision 1
# baseline (speedup 1.0000x reference)
"""MLA (multi-head latent attention) Bass kernel for Trainium2, 8 NeuronCores.

Sharding: batch (2) x head-group (4 groups of 4 heads) = 8 cores.
Each core computes, for its batch b and head group g:
  - latents  qlatT = (x_b @ wq_down)^T, kvlatT = (x_b @ wkv_down)^T  (replicated per batch)
  - k_rope   (shared across heads, replicated)
  - up-projections for its 4 heads, causal attention, and a partial
    output  out_partial = ctx_g @ wo[512g:512(g+1), :].
Host sums the 4 partial outputs per batch (the wo row-shard reduction).

All matmuls run in bf16 with fp32 PSUM accumulation. Softmax skips the
row-max subtraction (scores are O(+-10), exp stays in fp32 range).
"""
import math
import sys

sys.path.insert(0, "/opt/trn_rl_repo")

import numpy as np
import ml_dtypes

B, L, H = 2, 2048, 2048
NH, HD, RD = 16, 128, 64
QR, KVR = 768, 512
NHC = 4            # heads per core
N_CORES = 8
SCALE = 1.0 / math.sqrt(HD + RD)
BF = ml_dtypes.bfloat16

_NC_CACHE = {}


def build_nc(l_tokens=L, reps=1, phases=(1, 2, 3)):
    """Build + compile the per-core Bass program (parametrized by sequence
    length for small-scale testing; the real kernel uses l_tokens=L).
    reps>1 repeats the whole computation for timing-slope measurements."""
    import concourse.bass as bass  # noqa: F401
    import concourse.tile as tile
    from concourse import bacc, mybir

    dt = mybir.dt
    Lk = l_tokens
    assert Lk % 512 == 0
    TB = Lk // 512          # 512-token blocks
    KC = Lk // 128          # 128-token chunks
    HB = H // 512           # output column blocks

    nc = bacc.Bacc("TRN2", target_bir_lowering=False, debug=False,
                   num_devices=N_CORES)

    def din(name, shape, d=dt.bfloat16):
        return nc.dram_tensor(name, shape, d, kind="ExternalInput").ap()

    xTq = din("xTq", [H, 512])      # own 512-token block of x^T
    wqd = din("wqd", [H, QR])
    wkvd = din("wkvd", [H, KVR])
    wkr = din("wkr", [H, RD])
    wqu = din("wqu", [QR, NHC * HD])
    wqr = din("wqr", [QR, NHC * RD])
    wku = din("wku", [KVR, NHC * HD])
    wvu = din("wvu", [KVR, NHC * HD])
    wo = din("wo", [NHC * HD, H])
    cosT = din("cosT", [Lk, RD], dt.float32)
    ssT = din("ssT", [Lk, RD], dt.float32)     # [-sin | +sin]
    cosq = din("cosq", [512, RD], dt.float32)  # own-block slices
    ssq = din("ssq", [512, RD], dt.float32)
    maskm = din("maskm", [4, 128, 512])        # multiplicative causal masks
    ident = din("ident", [128, 128])
    out = nc.dram_tensor("out", [Lk, H], dt.bfloat16, kind="ExternalOutput").ap()

    with tile.TileContext(nc) as tc:
        with (
            tc.tile_pool(name="const", bufs=1) as cpool,
            tc.tile_pool(name="attn", bufs=1) as apool,
            tc.tile_pool(name="stream", bufs=2) as spool,
            tc.tile_pool(name="wcolp", bufs=3) as wpool,
            tc.tile_pool(name="ptp", bufs=4) as ptpool,
            tc.tile_pool(name="obp", bufs=4) as opool,
            tc.tile_pool(name="recp", bufs=1) as rpool,
            tc.tile_pool(name="xbp", bufs=1) as xbpool,
            tc.tile_pool(name="psA", bufs=2, space="PSUM") as psA,      # [128,512] f32
            tc.tile_pool(name="psSc", bufs=2, space="PSUM") as psSc,    # attn scores
            tc.tile_pool(name="psSmall", bufs=1, space="PSUM") as psS,  # small f32
            tc.tile_pool(name="psTp", bufs=1, space="PSUM") as psT,     # bf16 transposes
            tc.tile_pool(name="psCtx", bufs=1, space="PSUM") as psC,
            tc.tile_pool(name="psSum", bufs=1, space="PSUM") as psM,
        ):
            # ---- constants (DMAs for most are emitted lazily after the
            # first latent matmuls so the PE can start ~2.5MB into the DMA
            # stream instead of ~8MB) ----
            wqu_sb = cpool.tile([128, QR // 128, 512], dt.bfloat16, name="wqu_sb")
            wqr_sb = cpool.tile([128, QR // 128, 256], dt.bfloat16, name="wqr_sb")
            wku_sb = cpool.tile([128, KVR // 128, 512], dt.bfloat16, name="wku_sb")
            wvu_sb = cpool.tile([128, KVR // 128, 512], dt.bfloat16, name="wvu_sb")
            wo_sb = cpool.tile([128, NHC, H], dt.bfloat16, name="wo_sb")
            wkr_sb = cpool.tile([128, H // 128, RD], dt.bfloat16, name="wkr_sb")
            cos_sb = cpool.tile([128, KC, RD], dt.float32, name="cos_sb")
            ss_sb = cpool.tile([128, KC, RD], dt.float32, name="ss_sb")
            mask_sb = cpool.tile([128, 4, 512], dt.bfloat16, name="mask_sb")
            id_sb = cpool.tile([128, 128], dt.bfloat16, name="id_sb")
            ones_sb = cpool.tile([128, 1], dt.bfloat16, name="ones_sb")
            cosq_sb = cpool.tile([128, 4, RD], dt.float32, name="cosq_sb")
            ssq_sb = cpool.tile([128, 4, RD], dt.float32, name="ssq_sb")

            def load_deferred_consts():
                # ordered by first use: qcT needs wqu, kcT wku, v wvu, rope
                # kr/cos/ss/ident, q-rope wqr; attention masks last
                nc.sync.dma_start(wqu_sb, wqu.rearrange("(m p) n -> p m n", p=128))
                nc.sync.dma_start(wku_sb, wku.rearrange("(m p) n -> p m n", p=128))
                nc.sync.dma_start(wvu_sb, wvu.rearrange("(m p) n -> p m n", p=128))
                nc.sync.dma_start(cos_sb, cosT.rearrange("(c p) d -> p c d", p=128))
                nc.sync.dma_start(ss_sb, ssT.rearrange("(c p) d -> p c d", p=128))
                nc.sync.dma_start(wqr_sb, wqr.rearrange("(m p) n -> p m n", p=128))
                nc.sync.dma_start(mask_sb, maskm.rearrange("m p j -> p m j"))
                nc.vector.memset(ones_sb, 1.0)

            # ---- persistent attention operands ----
            qcT_sb = apool.tile([128, NHC, Lk], dt.bfloat16, name="qcT_sb")
            kcT_sb = apool.tile([128, NHC, Lk], dt.bfloat16, name="kcT_sb")
            qrT_sb = apool.tile([128, 2, Lk], dt.bfloat16, name="qrT_sb")
            krT_sb = apool.tile([128, Lk], dt.bfloat16, name="krT_sb")  # duplicated rows
            v_sb = apool.tile([128, KC, 512], dt.bfloat16, name="v_sb")
            ctxTn_sb = apool.tile([128, NHC, Lk], dt.bfloat16, name="ctxTn_sb")

            # ================= Phase 1: projections =================
            # Each core computes latents (q_latent, kv_latent, roped k_rope)
            # ONLY for its own 512-token block (per-core input xTq), then an
            # intra-batch AllGather (cores 0-3 / 4-7) assembles the full-L
            # latents, from which each core runs its own-head up-projections.
            CCG = [[0, 1, 2, 3], [4, 5, 6, 7]]
            NQ, NKV = QR // 128, KVR // 128
            gkv_rows = KVR + 128      # kv latent + roped/dup'd krT block
            skv_d = nc.dram_tensor("skv_d", [gkv_rows, 512], dt.bfloat16).ap()
            sql_d = nc.dram_tensor("sql_d", [QR, 512], dt.bfloat16).ap()
            if TB > 1:
                gkv_d = nc.dram_tensor("gkv_d", [TB * gkv_rows, 512],
                                       dt.bfloat16).ap()
                gql_d = nc.dram_tensor("gql_d", [TB * QR, 512],
                                       dt.bfloat16).ap()
            else:
                gkv_d, gql_d = skv_d, sql_d

            for _rep in range(reps):
              if 1 not in phases and _rep == 0:
                  load_deferred_consts()
                  for t_ in (qcT_sb, kcT_sb, qrT_sb, krT_sb, v_sb, ctxTn_sb):
                      nc.vector.memset(t_, 0.001)
              if 1 in phases:
                # ---- Phase 1a: own-block latents ----
                xb = xbpool.tile([128, H // 128, 512], dt.bfloat16, tag="xb")
                for m in range(NKV):
                    wc = wpool.tile([128, H // 128, 128], dt.bfloat16, tag="wcol")
                    if m == 0:
                        first = (_rep == 0)
                        if first:
                            nc.sync.dma_start(
                                wkr_sb, wkr.rearrange("(k p) d -> p k d", p=128))
                            nc.sync.dma_start(id_sb, ident)
                            nc.sync.dma_start(
                                cosq_sb, cosq.rearrange("(c p) d -> p c d", p=128))
                            nc.sync.dma_start(
                                ssq_sb, ssq.rearrange("(c p) d -> p c d", p=128))
                        for k in range(H // 128):
                            nc.sync.dma_start(
                                xb[:, k, :], xTq[k * 128:(k + 1) * 128, :])
                            if first:
                                nc.sync.dma_start(
                                    wc[:, k, :],
                                    wkvd[k * 128:(k + 1) * 128, 0:128])
                        if not first:
                            nc.sync.dma_start(
                                wc, wkvd[:, 0:128]
                                .rearrange("(k p) m -> p k m", p=128))
                    else:
                        nc.sync.dma_start(
                            wc, wkvd[:, m * 128:(m + 1) * 128]
                            .rearrange("(k p) m -> p k m", p=128))
                    ps = psA.tile([128, 512], dt.float32, tag="mm")
                    for k in range(H // 128):
                        nc.tensor.matmul(ps, wc[:, k, :], xb[:, k, :],
                                         start=(k == 0), stop=(k == H // 128 - 1))
                    lt = spool.tile([128, 512], dt.bfloat16, tag="lat")
                    nc.scalar.copy(lt, ps)
                    nc.sync.dma_start(skv_d[m * 128:(m + 1) * 128, :], lt)

                # roped k_rope for own block (feature-on-partition, duplicated)
                for tc2 in range(4):
                    tsl = slice(tc2 * 128, (tc2 + 1) * 128)
                    kr_ps = psS.tile([128, RD], dt.float32, tag="sm")
                    for k in range(H // 128):
                        nc.tensor.matmul(kr_ps, xb[:, k, tsl], wkr_sb[:, k, :],
                                         start=(k == 0), stop=(k == H // 128 - 1))
                    t1 = spool.tile([128, RD], dt.float32, tag="t1")
                    nc.vector.tensor_tensor(t1, kr_ps, cosq_sb[:, tc2, :],
                                            mybir.AluOpType.mult)
                    t2 = spool.tile([128, RD], dt.float32, tag="t2")
                    nc.vector.tensor_tensor(t2[:, 0:32], kr_ps[:, 32:64],
                                            ssq_sb[:, tc2, 0:32],
                                            mybir.AluOpType.mult)
                    nc.vector.tensor_tensor(t2[:, 32:64], kr_ps[:, 0:32],
                                            ssq_sb[:, tc2, 32:64],
                                            mybir.AluOpType.mult)
                    krb = spool.tile([128, 128], dt.bfloat16, tag="krb")
                    nc.vector.tensor_tensor(krb[:, 0:64], t1, t2,
                                            mybir.AluOpType.add)
                    nc.vector.tensor_copy(krb[:, 64:128], krb[:, 0:64])
                    ktp = psT.tile([128, 128], dt.bfloat16, tag="tp")
                    nc.tensor.transpose(ktp, krb, id_sb)
                    kt = spool.tile([128, 128], dt.bfloat16, tag="krt")
                    nc.vector.tensor_copy(kt, ktp)
                    nc.sync.dma_start(skv_d[KVR:KVR + 128, tsl], kt)

                if TB > 1:
                    nc.gpsimd.collective_compute(
                        "AllGather", mybir.AluOpType.bypass,
                        replica_groups=CCG, ins=[skv_d], outs=[gkv_d])

                # q latent for own block
                for m in range(NQ):
                    wc = wpool.tile([128, H // 128, 128], dt.bfloat16, tag="wcol")
                    nc.sync.dma_start(
                        wc, wqd[:, m * 128:(m + 1) * 128]
                        .rearrange("(k p) m -> p k m", p=128))
                    ps = psA.tile([128, 512], dt.float32, tag="mm")
                    for k in range(H // 128):
                        nc.tensor.matmul(ps, wc[:, k, :], xb[:, k, :],
                                         start=(k == 0), stop=(k == H // 128 - 1))
                    lt = spool.tile([128, 512], dt.bfloat16, tag="lat")
                    nc.scalar.copy(lt, ps)
                    nc.sync.dma_start(sql_d[m * 128:(m + 1) * 128, :], lt)

                if TB > 1:
                    nc.gpsimd.collective_compute(
                        "AllGather", mybir.AluOpType.bypass,
                        replica_groups=CCG, ins=[sql_d], outs=[gql_d])

                if _rep == 0:
                    load_deferred_consts()

                # ---- Phase 1b: per-block up-projections from gathered latents
                for tb in range(TB):
                  ts0 = tb * 512

                  # full-L krT directly from the gathered shard
                  nc.sync.dma_start(
                      krT_sb[:, ts0:ts0 + 512],
                      gkv_d[tb * gkv_rows + KVR:tb * gkv_rows + KVR + 128, :])

                  kvb = spool.tile([128, NKV, 512], dt.bfloat16, tag="kvb")
                  nc.sync.dma_start(
                      kvb, gkv_d[tb * gkv_rows:tb * gkv_rows + KVR, :]
                      .rearrange("(m p) t -> p m t", p=128))

                  # kcT
                  for hc in range(NHC):
                      ps = psA.tile([128, 512], dt.float32, tag="mm")
                      for m in range(NKV):
                          nc.tensor.matmul(ps,
                                           wku_sb[:, m, hc * 128:(hc + 1) * 128],
                                           kvb[:, m, :],
                                           start=(m == 0), stop=(m == NKV - 1))
                      nc.scalar.copy(kcT_sb[:, hc, ts0:ts0 + 512], ps)

                  # v (token-on-partition)
                  for tc2 in range(4):
                      ps = psA.tile([128, 512], dt.float32, tag="mm")
                      for m in range(NKV):
                          nc.tensor.matmul(ps,
                                           kvb[:, m, tc2 * 128:(tc2 + 1) * 128],
                                           wvu_sb[:, m, :],
                                           start=(m == 0), stop=(m == NKV - 1))
                      nc.scalar.copy(v_sb[:, tb * 4 + tc2, :], ps)

                  qlb = spool.tile([128, NQ, 512], dt.bfloat16, tag="qlb")
                  nc.sync.dma_start(
                      qlb, gql_d[tb * QR:(tb + 1) * QR, :]
                      .rearrange("(m p) t -> p m t", p=128))

                  # qcT
                  for hc in range(NHC):
                      ps = psA.tile([128, 512], dt.float32, tag="mm")
                      for m in range(NQ):
                          nc.tensor.matmul(ps,
                                           wqu_sb[:, m, hc * 128:(hc + 1) * 128],
                                           qlb[:, m, :],
                                           start=(m == 0), stop=(m == NQ - 1))
                      nc.scalar.copy(qcT_sb[:, hc, ts0:ts0 + 512], ps)

                  # q_rope per 128-token chunk
                  for tc2 in range(4):
                    gc = tb * 4 + tc2
                    tsl = slice(tc2 * 128, (tc2 + 1) * 128)
                    qr_ps = psS.tile([128, NHC * RD], dt.float32, tag="sm")
                    for m in range(NQ):
                        nc.tensor.matmul(qr_ps, qlb[:, m, tsl], wqr_sb[:, m, :],
                                         start=(m == 0), stop=(m == NQ - 1))
                    qrv = qr_ps.rearrange("p (h d) -> p h d", d=RD)
                    q1 = spool.tile([128, NHC, RD], dt.float32, tag="q1")
                    nc.vector.tensor_tensor(
                        q1, qrv,
                        cos_sb[:, gc, None, :].to_broadcast([128, NHC, RD]),
                        mybir.AluOpType.mult)
                    q2 = spool.tile([128, NHC, RD], dt.float32, tag="q2")
                    nc.vector.tensor_tensor(
                        q2[:, :, 0:32], qrv[:, :, 32:64],
                        ss_sb[:, gc, None, 0:32].to_broadcast([128, NHC, 32]),
                        mybir.AluOpType.mult)
                    nc.vector.tensor_tensor(
                        q2[:, :, 32:64], qrv[:, :, 0:32],
                        ss_sb[:, gc, None, 32:64].to_broadcast([128, NHC, 32]),
                        mybir.AluOpType.mult)
                    qrb = spool.tile([128, NHC * RD], dt.bfloat16, tag="qrb")
                    nc.vector.tensor_tensor(
                        qrb.rearrange("p (h d) -> p h d", d=RD), q1, q2,
                        mybir.AluOpType.add)
                    for hp in range(2):
                        qtp = psT.tile([128, 128], dt.bfloat16, tag="tp")
                        nc.tensor.transpose(qtp, qrb[:, hp * 128:(hp + 1) * 128],
                                            id_sb)
                        nc.vector.tensor_copy(
                            qrT_sb[:, hp, gc * 128:(gc + 1) * 128], qtp)

              # ============ Phase 2+3: attention, WO interleaved ============
              # WO for query-block s is emitted right after attention block s,
              # so its matmuls fill the next block's exp-latency PE gaps
              # instead of piling up into a copy-chain-bound tail.
              if _rep == 0:
                  nc.sync.dma_start(wo_sb, wo.rearrange("(h p) n -> p h n", p=128))

              def emit_wo_block(s):
                  for tc3 in range(4 * s, 4 * s + 4):
                      csl = slice(tc3 * 128, (tc3 + 1) * 128)
                      for nb in range(HB):
                          nsl = slice(nb * 512, (nb + 1) * 512)
                          po = psA.tile([128, 512], dt.float32, tag="mm")
                          for h in range(NHC):
                              nc.tensor.matmul(po, ctxTn_sb[:, h, csl],
                                               wo_sb[:, h, nsl],
                                               start=(h == 0), stop=(h == NHC - 1))
                          ob = opool.tile([128, 512], dt.bfloat16, tag="ob")
                          nc.vector.tensor_copy(ob, po)
                          nc.sync.dma_start(out[csl, nsl], ob)

              for s in range(TB if 2 in phases else 0):
                  qsl = slice(s * 512, (s + 1) * 512)
                  if s > 0 and 3 in phases:
                      emit_wo_block(s - 1)
                  for h in range(NHC):
                      hp, half = divmod(h, 2)
                      base = 64 * half
                      nck = 4 * s + 4
                      ctx_ps = psC.tile([128, 512], dt.float32, tag="ctx")
                      sum_ps = psM.tile([1, 512], dt.float32, tag="sum")

                      def emit_pv(pt, c, nck=nck, h=h, sum_ps=sum_ps,
                                  ctx_ps=ctx_ps):
                          nc.tensor.matmul(sum_ps, ones_sb, pt,
                                           start=(c == 0), stop=(c == nck - 1))
                          nc.tensor.matmul(ctx_ps,
                                           v_sb[:, c, h * 128:(h + 1) * 128],
                                           pt, start=(c == 0), stop=(c == nck - 1))

                      pending = []
                      for c in range(nck):
                          ksl = slice(c * 128, (c + 1) * 128)
                          diag = (c // 4 == s)
                          off = 128 * (c % 4) if diag else 0
                          qs2 = slice(s * 512 + off, (s + 1) * 512)
                          sc = psSc.tile([128, 512], dt.float32, tag="sc")
                          nc.tensor.matmul(sc[:, off:], kcT_sb[:, h, ksl],
                                           qcT_sb[:, h, qs2],
                                           start=True, stop=False)
                          nc.tensor.matmul(
                              sc[:, off:],
                              krT_sb[base:base + 64, ksl],
                              qrT_sb[base:base + 64, hp, qs2],
                              start=False, stop=True)
                          pt = ptpool.tile([128, 512], dt.bfloat16, tag="pt")
                          if off:
                              nc.vector.memset(pt[:, 0:off], 0.0)
                          nc.scalar.activation(pt[:, off:], sc[:, off:],
                                               mybir.ActivationFunctionType.Exp,
                                               scale=SCALE)
                          if diag:
                              nc.vector.tensor_tensor(
                                  pt[:, off:off + 128], pt[:, off:off + 128],
                                  mask_sb[:, 0, 0:128], mybir.AluOpType.mult)
                          # software pipeline (lag 2): sum/PV trail the scores
                          # by two iterations so the PE rides over the exp
                          # latency of each block
                          pending.append((pt, c))
                          if len(pending) > 2:
                              emit_pv(*pending.pop(0))
                      for p in pending:
                          emit_pv(*p)
                      rec = rpool.tile([1, 512], dt.float32, tag="rec")
                      nc.vector.reciprocal(rec, sum_ps)
                      rb = rpool.tile([128, 512], dt.float32, tag="rb")
                      nc.gpsimd.partition_broadcast(rb, rec)
                      nc.vector.tensor_tensor(ctxTn_sb[:, h, qsl], ctx_ps, rb,
                                              mybir.AluOpType.mult)

              # final WO block (query block TB-1)
              if 3 in phases:
                  if 2 in phases:
                      emit_wo_block(TB - 1)
                  else:
                      for s_ in range(TB):
                          emit_wo_block(s_)

    nc.compile()
    return nc


def _host_tables(l_tokens):
    inv_freq = (1.0 / (10000.0 ** (np.arange(0, RD, 2, dtype=np.float32) / RD))
                ).astype(np.float32)
    pos = np.arange(l_tokens, dtype=np.float32)
    freqs = np.outer(pos, inv_freq).astype(np.float32)
    cos_t = np.concatenate([np.cos(freqs), np.cos(freqs)], axis=-1)
    ss_t = np.concatenate([-np.sin(freqs), np.sin(freqs)], axis=-1)
    return cos_t.astype(np.float32), ss_t.astype(np.float32)


def _host_masks():
    r = np.arange(128)[:, None]
    j = np.arange(512)[None, :]
    m = np.stack([(j >= 128 * mm + r) for mm in range(4)]).astype(np.float32)
    return m.astype(BF)


def make_in_maps(inputs, l_tokens=L):
    """Build the 8 per-core input maps from the full (unsharded) inputs."""
    x = np.asarray(inputs["x"], np.float32)
    cos_t, ss_t = _host_tables(l_tokens)
    maskm = _host_masks()
    ident = np.eye(128, dtype=np.float32).astype(BF)

    xTs = [np.ascontiguousarray(x[b, :l_tokens].T).astype(BF) for b in range(x.shape[0])]
    n_blk = l_tokens // 512
    wqd = np.asarray(inputs["wq_down"], np.float32).astype(BF)
    wkvd = np.asarray(inputs["wkv_down"], np.float32).astype(BF)
    wkr = np.asarray(inputs["wk_rope"], np.float32).astype(BF)
    wqu = np.asarray(inputs["wq_up"], np.float32).astype(BF)
    wqr = np.asarray(inputs["wq_rope"], np.float32).astype(BF)
    wku = np.asarray(inputs["wk_up"], np.float32).astype(BF)
    wvu = np.asarray(inputs["wv_up"], np.float32).astype(BF)
    wo = np.asarray(inputs["wo"], np.float32).astype(BF)

    in_maps = []
    for core in range(N_CORES):
        b, g = divmod(core, 4)
        blk = g % n_blk
        in_maps.append({
            "xTq": np.ascontiguousarray(xTs[b][:, blk * 512:(blk + 1) * 512]),
            "cosq": np.ascontiguousarray(cos_t[blk * 512:(blk + 1) * 512]),
            "ssq": np.ascontiguousarray(ss_t[blk * 512:(blk + 1) * 512]),
            "wqd": wqd,
            "wkvd": wkvd,
            "wkr": wkr,
            "wqu": np.ascontiguousarray(wqu[:, g * 512:(g + 1) * 512]),
            "wqr": np.ascontiguousarray(wqr[:, g * 256:(g + 1) * 256]),
            "wku": np.ascontiguousarray(wku[:, g * 512:(g + 1) * 512]),
            "wvu": np.ascontiguousarray(wvu[:, g * 512:(g + 1) * 512]),
            "wo": np.ascontiguousarray(wo[g * 512:(g + 1) * 512, :]),
            "cosT": cos_t,
            "ssT": ss_t,
            "maskm": maskm,
            "ident": ident,
        })
    return in_maps


def kernel(**inputs):
    from concourse.bass_utils import run_bass_kernel_spmd

    if L not in _NC_CACHE:
        _NC_CACHE[L] = build_nc(L)
    nc = _NC_CACHE[L]
    in_maps = make_in_maps(inputs, L)
    res = run_bass_kernel_spmd(nc, in_maps, list(range(N_CORES)))
    out = np.zeros((B, L, H), np.float32)
    for core in range(N_CORES):
        b, _g = divmod(core, 4)
        out[b] += res.results[core]["out"].astype(np.float32)
    return out



# revision 19
# speedup vs baseline: 1.2737x; 1.2737x over previous
"""MLA (multi-head latent attention) Bass kernel for Trainium2, 8 NeuronCores.

Sharding: batch (2) x head-group (4 groups of 4 heads) = 8 cores.
Each core, for its batch b and head group g (pair position p = g % 2):
  - computes kv latent + roped k_rope for its own 512-token block, then ONE
    intra-batch AllGather (cores 0-3 / 4-7) assembles the full-L kv latents
  - computes q latents for token blocks {p, p+2}; two pairwise AllGathers
    (cores {0,1},{2,3},...) assemble blocks {0,1} and {2,3} -- the gathered
    buffers hold blocks in a core-independent order, keeping the program
    SPMD-uniform
  - up-projects q/k/v for its 4 heads, runs causal attention, and a partial
    output  out_partial = ctx_g @ wo[512g:512(g+1), :].
Host sums the 4 partial outputs per batch (the wo row-shard reduction).

DMA dispatch is spread across engine queues (sync/scalar/gpsimd) so
descriptor generation never serializes behind dependent stores; loads that
depend on a collective ride the gpsimd queue right behind that collective.

All matmuls run in bf16 with fp32 PSUM accumulation. Softmax skips the
row-max subtraction (scores are O(+-10), exp stays in fp32 range).
"""
import math
import sys

sys.path.insert(0, "/opt/trn_rl_repo")

import numpy as np
import ml_dtypes

B, L, H = 2, 2048, 2048
NH, HD, RD = 16, 128, 64
QR, KVR = 768, 512
NHC = 4            # heads per core
N_CORES = 8
SCALE = 1.0 / math.sqrt(HD + RD)
BF = ml_dtypes.bfloat16

_NC_CACHE = {}


def build_nc(l_tokens=L):
    import concourse.bass as bass  # noqa: F401
    import concourse.tile as tile
    from concourse import bacc, mybir

    dt = mybir.dt
    Lk = l_tokens
    assert Lk % 512 == 0
    TB = Lk // 512          # 512-token blocks
    assert TB in (1, 4)
    KC = Lk // 128          # 128-token chunks
    HB = H // 512           # output column blocks
    NQ, NKV = QR // 128, KVR // 128
    HK = H // 128
    NQB = 1 if TB == 1 else 2   # own q-latent blocks per core

    nc = bacc.Bacc("TRN2", target_bir_lowering=False, debug=False,
                   num_devices=N_CORES)

    def din(name, shape, d=dt.bfloat16):
        return nc.dram_tensor(name, shape, d, kind="ExternalInput").ap()

    xq = din("xq", [H, 512])          # own 512-token block of x^T
    xq2 = din("xq2", [H, NQB * 512])  # q-latent blocks {p, p+2} of x^T
    wqd = din("wqd", [H, QR])
    wkvd = din("wkvd", [H, KVR])
    wkr = din("wkr", [H, RD])
    wqu = din("wqu", [QR, NHC * HD])
    wqr = din("wqr", [QR, NHC * RD])
    wku = din("wku", [KVR, NHC * HD])
    wvu = din("wvu", [KVR, NHC * HD])
    wo = din("wo", [NHC * HD, H])
    cosT = din("cosT", [Lk, RD], dt.float32)
    ssT = din("ssT", [Lk, RD], dt.float32)     # [-sin | +sin]
    cosq = din("cosq", [512, RD], dt.float32)  # own-block slices
    ssq = din("ssq", [512, RD], dt.float32)
    maskm = din("maskm", [4, 128, 512])        # multiplicative causal masks
    ident = din("ident", [128, 128])
    out = nc.dram_tensor("out", [Lk, H], dt.bfloat16, kind="ExternalOutput").ap()

    # kv gather: KVR latent rows + RD krope-T rows per block
    GR = KVR + RD
    CCG = [[0, 1, 2, 3], [4, 5, 6, 7]]
    CCP = [[0, 1], [2, 3], [4, 5], [6, 7]]
    skv_d = nc.dram_tensor("skv_d", [GR, 512], dt.bfloat16).ap()
    sql_a = nc.dram_tensor("sql_a", [QR, 512], dt.bfloat16).ap()
    if TB > 1:
        gkv_d = nc.dram_tensor("gkv_d", [TB * GR, 512], dt.bfloat16).ap()
        sql_b = nc.dram_tensor("sql_b", [QR, 512], dt.bfloat16).ap()
        gql_a = nc.dram_tensor("gql_a", [2 * QR, 512], dt.bfloat16).ap()
        gql_b = nc.dram_tensor("gql_b", [2 * QR, 512], dt.bfloat16).ap()
    else:
        gkv_d = skv_d
        sql_b = gql_a = gql_b = sql_a

    with tile.TileContext(nc) as tc:
        with (
            tc.tile_pool(name="const", bufs=1) as cpool,
            tc.tile_pool(name="attn", bufs=1) as apool,
        ):
            # ---- whole-program constants ----
            wkr_sb = cpool.tile([128, HK, RD], dt.bfloat16, name="wkr_sb")
            cosq_sb = cpool.tile([128, 4, RD], dt.float32, name="cosq_sb")
            ssq_sb = cpool.tile([128, 4, RD], dt.float32, name="ssq_sb")
            id_sb = cpool.tile([128, 128], dt.bfloat16, name="id_sb")
            cos_sb = cpool.tile([128, KC, RD], dt.float32, name="cos_sb")
            ss_sb = cpool.tile([128, KC, RD], dt.float32, name="ss_sb")

            # ---- persistent attention operands ----
            qcT_sb = apool.tile([128, NHC, Lk], dt.bfloat16, name="qcT_sb")
            kcT_sb = apool.tile([128, NHC, Lk], dt.bfloat16, name="kcT_sb")
            qrT_sb = apool.tile([128, 2, Lk], dt.bfloat16, name="qrT_sb")
            krT_sb = apool.tile([128, Lk], dt.bfloat16, name="krT_sb")
            v_sb = apool.tile([128, KC, 512], dt.bfloat16, name="v_sb")

            # ================= Phase 1: projections =================
            with (
                tc.tile_pool(name="p1w", bufs=1) as wpool,
                tc.tile_pool(name="qwcp", bufs=NQ) as qwcpool,
                tc.tile_pool(name="wcolp", bufs=2) as wcpool,
                tc.tile_pool(name="xbp", bufs=2) as xpool,
                tc.tile_pool(name="qlp", bufs=2) as qlpool,
                tc.tile_pool(name="kvbp", bufs=3) as kvbpool,
                tc.tile_pool(name="p1s", bufs=2) as spool,
                tc.tile_pool(name="psA", bufs=2, space="PSUM") as psA,
                tc.tile_pool(name="psSmall", bufs=2, space="PSUM") as psS,
                tc.tile_pool(name="psTp", bufs=2, space="PSUM") as psT,
            ):
                # xb ring: xb0 first so that the third allocation (xqb1)
                # recycles xb0's buffer, whose readers are all in phase 1a
                xb0 = xpool.tile([128, HK, 512], dt.bfloat16, tag="xb")
                # --- prefetches on the SCALAR queue: q-latent inputs ---
                xqb0 = xpool.tile([128, HK, 512], dt.bfloat16, tag="xb")
                nc.scalar.dma_start(
                    xqb0, xq2[:, 0:512].rearrange("(k p) t -> p k t", p=128))
                qwc = []
                for m in range(NQ):
                    wc = qwcpool.tile([128, HK, 128], dt.bfloat16, tag="qwc")
                    nc.scalar.dma_start(
                        wc, wqd[:, m * 128:(m + 1) * 128]
                        .rearrange("(k p) m -> p k m", p=128))
                    qwc.append(wc)

                # --- 1a: own-block kv latent + roped k_rope (sync queue) ---
                for m in range(NKV):
                    wc = wcpool.tile([128, HK, 128], dt.bfloat16, tag="wcol")
                    nc.sync.dma_start(
                        wc, wkvd[:, m * 128:(m + 1) * 128]
                        .rearrange("(k p) m -> p k m", p=128))
                    if m == 0:
                        for piece in range(4):
                            ks = slice(piece * 4 * 128, (piece + 1) * 4 * 128)
                            nc.sync.dma_start(
                                xb0[:, piece * 4:(piece + 1) * 4, :],
                                xq[ks, :].rearrange("(k p) t -> p k t", p=128))
                        nc.sync.dma_start(
                            wkr_sb, wkr.rearrange("(k p) d -> p k d", p=128))
                        nc.sync.dma_start(
                            cosq_sb, cosq.rearrange("(c p) d -> p c d", p=128))
                        nc.sync.dma_start(
                            ssq_sb, ssq.rearrange("(c p) d -> p c d", p=128))
                        nc.sync.dma_start(id_sb, ident)
                        wku_sb = wpool.tile([128, NKV, 512], dt.bfloat16,
                                            name="wku_sb")
                        nc.sync.dma_start(
                            wku_sb, wku.rearrange("(m p) n -> p m n", p=128))
                        wvu_sb = wpool.tile([128, NKV, 512], dt.bfloat16,
                                            name="wvu_sb")
                        nc.sync.dma_start(
                            wvu_sb, wvu.rearrange("(m p) n -> p m n", p=128))
                        wqu_sb = wpool.tile([128, NQ, 512], dt.bfloat16,
                                            name="wqu_sb")
                        nc.sync.dma_start(
                            wqu_sb, wqu.rearrange("(m p) n -> p m n", p=128))
                        wqr_sb = wpool.tile([128, NQ, 256], dt.bfloat16,
                                            name="wqr_sb")
                        nc.sync.dma_start(
                            wqr_sb, wqr.rearrange("(m p) n -> p m n", p=128))
                        nc.sync.dma_start(
                            cos_sb, cosT.rearrange("(c p) d -> p c d", p=128))
                        nc.sync.dma_start(
                            ss_sb, ssT.rearrange("(c p) d -> p c d", p=128))
                    ps = psA.tile([128, 512], dt.float32, tag="mm")
                    for k in range(HK):
                        nc.tensor.matmul(ps, wc[:, k, :], xb0[:, k, :],
                                         start=(k == 0), stop=(k == HK - 1))
                    lt = spool.tile([128, 512], dt.bfloat16, tag="lat")
                    nc.scalar.copy(lt, ps)
                    nc.sync.dma_start(skv_d[m * 128:(m + 1) * 128, :], lt)

                # roped k_rope for own block (feature-on-partition, 64 rows)
                for tc2 in range(4):
                    tsl = slice(tc2 * 128, (tc2 + 1) * 128)
                    kr_ps = psS.tile([128, RD], dt.float32, tag="sm")
                    for k in range(HK):
                        nc.tensor.matmul(kr_ps, xb0[:, k, tsl], wkr_sb[:, k, :],
                                         start=(k == 0), stop=(k == HK - 1))
                    t1 = spool.tile([128, RD], dt.float32, tag="t1")
                    nc.vector.tensor_tensor(t1, kr_ps, cosq_sb[:, tc2, :],
                                            mybir.AluOpType.mult)
                    t2 = spool.tile([128, RD], dt.float32, tag="t2")
                    nc.vector.tensor_tensor(t2[:, 0:32], kr_ps[:, 32:64],
                                            ssq_sb[:, tc2, 0:32],
                                            mybir.AluOpType.mult)
                    nc.vector.tensor_tensor(t2[:, 32:64], kr_ps[:, 0:32],
                                            ssq_sb[:, tc2, 32:64],
                                            mybir.AluOpType.mult)
                    krb = spool.tile([128, RD], dt.bfloat16, tag="krb")
                    nc.vector.tensor_tensor(krb, t1, t2, mybir.AluOpType.add)
                    ktp = psT.tile([128, 128], dt.bfloat16, tag="tp")
                    nc.tensor.transpose(ktp[0:RD, :], krb, id_sb)
                    kt = spool.tile([RD, 128], dt.bfloat16, tag="krt")
                    nc.vector.tensor_copy(kt, ktp[0:RD, :])
                    nc.sync.dma_start(skv_d[KVR:KVR + RD, tsl], kt)

                # --- CC1: gather kv latents across the 4-core batch group ---
                if TB > 1:
                    nc.gpsimd.collective_compute(
                        "AllGather", mybir.AluOpType.bypass,
                        replica_groups=CCG, ins=[skv_d], outs=[gkv_d])
                # block-0 kv load rides the gpsimd queue right behind CC1
                kvb0 = kvbpool.tile([128, NKV, 512], dt.bfloat16, tag="kvb")
                kvbs = [kvb0]
                nc.sync.dma_start(
                    kvb0,
                    gkv_d[0:KVR, :].rearrange("(m p) t -> p m t", p=128))

                # --- 1c: q latents for own blocks {p, p+2} ---
                for j in range(NQB):
                    if j == 1:
                        # second own block's x: recycles xb0's buffer (all
                        # of whose readers were emitted in phase 1a)
                        xqb1 = xpool.tile([128, HK, 512], dt.bfloat16,
                                          tag="xb")
                        nc.scalar.dma_start(
                            xqb1, xq2[:, 512:1024]
                            .rearrange("(k p) t -> p k t", p=128))
                        xjb = xqb1
                    else:
                        xjb = xqb0
                    sql = sql_a if j == 0 else sql_b
                    for m in range(NQ):
                        ps = psA.tile([128, 512], dt.float32, tag="mm")
                        for k in range(HK):
                            nc.tensor.matmul(ps, qwc[m][:, k, :], xjb[:, k, :],
                                             start=(k == 0), stop=(k == HK - 1))
                        lt = spool.tile([128, 512], dt.bfloat16, tag="lat")
                        nc.scalar.copy(lt, ps)
                        nc.sync.dma_start(sql[m * 128:(m + 1) * 128, :], lt)
                    if TB > 1:
                        nc.gpsimd.collective_compute(
                            "AllGather", mybir.AluOpType.bypass,
                            replica_groups=CCP, ins=[sql],
                            outs=[gql_a if j == 0 else gql_b])
                    if j == 0 and TB > 1:
                        # block-1 kv load rides behind CC2a
                        kvb1 = kvbpool.tile([128, NKV, 512], dt.bfloat16,
                                            tag="kvb")
                        kvbs.append(kvb1)
                        nc.sync.dma_start(
                            kvb1, gkv_d[GR:GR + KVR, :]
                            .rearrange("(m p) t -> p m t", p=128))

                # behind CC2b: block-2 kv + the full-L krT rows
                if TB > 1:
                    kvb2 = kvbpool.tile([128, NKV, 512], dt.bfloat16,
                                        tag="kvb")
                    kvbs.append(kvb2)
                    nc.sync.dma_start(
                        kvb2, gkv_d[2 * GR:2 * GR + KVR, :]
                        .rearrange("(m p) t -> p m t", p=128))
                for tb in range(TB):
                    ts0 = tb * 512
                    nc.sync.dma_start(
                        krT_sb[0:RD, ts0:ts0 + 512],
                        gkv_d[tb * GR + KVR:tb * GR + KVR + RD, :])
                    nc.sync.dma_start(
                        krT_sb[RD:2 * RD, ts0:ts0 + 512],
                        gkv_d[tb * GR + KVR:tb * GR + KVR + RD, :])

                # --- 1d: kcT + v from the gathered kv latents ---
                for tb in range(TB):
                    ts0 = tb * 512
                    kvb = kvbs[tb]
                    for hc in range(NHC):
                        ps = psA.tile([128, 512], dt.float32, tag="mm")
                        for m in range(NKV):
                            nc.tensor.matmul(ps,
                                             wku_sb[:, m, hc * 128:(hc + 1) * 128],
                                             kvb[:, m, :],
                                             start=(m == 0), stop=(m == NKV - 1))
                        nc.scalar.copy(kcT_sb[:, hc, ts0:ts0 + 512], ps)
                    for tc2 in range(4):
                        ps = psA.tile([128, 512], dt.float32, tag="mm")
                        for m in range(NKV):
                            nc.tensor.matmul(ps,
                                             kvb[:, m, tc2 * 128:(tc2 + 1) * 128],
                                             wvu_sb[:, m, :],
                                             start=(m == 0), stop=(m == NKV - 1))
                        nc.scalar.copy(v_sb[:, tb * 4 + tc2, :], ps)
                    if tb == 0 and TB > 1:
                        # block-3 kv load: safe to recycle kvbs[0]'s buffer
                        # now that block-0's readers are emitted
                        kvb3 = kvbpool.tile([128, NKV, 512], dt.bfloat16,
                                            tag="kvb")
                        kvbs.append(kvb3)
                        nc.sync.dma_start(
                            kvb3, gkv_d[3 * GR:3 * GR + KVR, :]
                            .rearrange("(m p) t -> p m t", p=128))

                # --- 1e: q up-projections from the gathered q latents ---
                for r in range(TB):
                    ts0 = r * 512
                    qlb = qlpool.tile([128, NQ, 512], dt.bfloat16, tag="qlb")
                    gq = gql_a if r < 2 else gql_b
                    row0 = (r % 2) * QR
                    nc.sync.dma_start(
                        qlb, gq[row0:row0 + QR, :]
                        .rearrange("(m p) t -> p m t", p=128))

                    for hc in range(NHC):
                        ps = psA.tile([128, 512], dt.float32, tag="mm")
                        for m in range(NQ):
                            nc.tensor.matmul(ps,
                                             wqu_sb[:, m, hc * 128:(hc + 1) * 128],
                                             qlb[:, m, :],
                                             start=(m == 0), stop=(m == NQ - 1))
                        nc.scalar.copy(qcT_sb[:, hc, ts0:ts0 + 512], ps)

                    for tc2 in range(4):
                        gc = r * 4 + tc2
                        tsl = slice(tc2 * 128, (tc2 + 1) * 128)
                        qr_ps = psS.tile([128, NHC * RD], dt.float32, tag="sm")
                        for m in range(NQ):
                            nc.tensor.matmul(qr_ps, qlb[:, m, tsl],
                                             wqr_sb[:, m, :],
                                             start=(m == 0), stop=(m == NQ - 1))
                        qrv = qr_ps.rearrange("p (h d) -> p h d", d=RD)
                        q1 = spool.tile([128, NHC, RD], dt.float32, tag="q1")
                        nc.vector.tensor_tensor(
                            q1, qrv,
                            cos_sb[:, gc, None, :].to_broadcast([128, NHC, RD]),
                            mybir.AluOpType.mult)
                        q2 = spool.tile([128, NHC, RD], dt.float32, tag="q2")
                        nc.vector.tensor_tensor(
                            q2[:, :, 0:32], qrv[:, :, 32:64],
                            ss_sb[:, gc, None, 0:32].to_broadcast([128, NHC, 32]),
                            mybir.AluOpType.mult)
                        nc.vector.tensor_tensor(
                            q2[:, :, 32:64], qrv[:, :, 0:32],
                            ss_sb[:, gc, None, 32:64].to_broadcast([128, NHC, 32]),
                            mybir.AluOpType.mult)
                        qrb = spool.tile([128, NHC * RD], dt.bfloat16, tag="qrb")
                        nc.vector.tensor_tensor(
                            qrb.rearrange("p (h d) -> p h d", d=RD), q1, q2,
                            mybir.AluOpType.add)
                        for hp in range(2):
                            qtp = psT.tile([128, 128], dt.bfloat16, tag="tp")
                            nc.tensor.transpose(
                                qtp, qrb[:, hp * 128:(hp + 1) * 128], id_sb)
                            nc.vector.tensor_copy(
                                qrT_sb[:, hp, gc * 128:(gc + 1) * 128], qtp)

            # ============ Phase 2+3: attention, WO interleaved ============
            # WO for query-block s is emitted right after attention block s,
            # so its matmuls fill the next block's exp-latency PE gaps
            # instead of piling up into a copy-chain-bound tail.
            with (
                tc.tile_pool(name="c2", bufs=1) as c2pool,
                tc.tile_pool(name="ptp", bufs=4) as ptpool,
                tc.tile_pool(name="obp", bufs=4) as opool,
                tc.tile_pool(name="recp", bufs=2) as rpool,
                tc.tile_pool(name="psSc", bufs=3, space="PSUM") as psSc,
                tc.tile_pool(name="psCtx", bufs=2, space="PSUM") as psC,
                tc.tile_pool(name="psSum", bufs=1, space="PSUM") as psM,
                tc.tile_pool(name="psWo", bufs=2, space="PSUM") as psW,
            ):
                ctxTn_sb = c2pool.tile([128, NHC, Lk], dt.bfloat16,
                                       name="ctxTn_sb")
                mask_sb = c2pool.tile([128, 4, 512], dt.bfloat16,
                                      name="mask_sb")
                nc.sync.dma_start(mask_sb, maskm.rearrange("m p j -> p m j"))
                ones_sb = c2pool.tile([128, 1], dt.bfloat16, name="ones_sb")
                nc.vector.memset(ones_sb, 1.0)
                wo_sb = c2pool.tile([128, NHC, H], dt.bfloat16, name="wo_sb")
                nc.sync.dma_start(wo_sb, wo.rearrange("(h p) n -> p h n", p=128))

                def emit_wo_block(s):
                    for tc3 in range(4 * s, 4 * s + 4):
                        csl = slice(tc3 * 128, (tc3 + 1) * 128)
                        for nb in range(HB):
                            nsl = slice(nb * 512, (nb + 1) * 512)
                            po = psW.tile([128, 512], dt.float32, tag="wo")
                            for h in range(NHC):
                                nc.tensor.matmul(po, ctxTn_sb[:, h, csl],
                                                 wo_sb[:, h, nsl],
                                                 start=(h == 0),
                                                 stop=(h == NHC - 1))
                            ob = opool.tile([128, 512], dt.bfloat16, tag="ob")
                            nc.vector.tensor_copy(ob, po)
                            nc.sync.dma_start(out[csl, nsl], ob)

                for s in range(TB):
                    qsl = slice(s * 512, (s + 1) * 512)
                    if s > 0:
                        emit_wo_block(s - 1)
                    for h in range(NHC):
                        hp, half = divmod(h, 2)
                        base = 64 * half
                        nck = 4 * s + 4
                        ctx_ps = psC.tile([128, 512], dt.float32, tag="ctx")
                        sum_ps = psM.tile([1, 512], dt.float32, tag="sum")

                        def emit_pv(pt, c, off, nck=nck, h=h, sum_ps=sum_ps,
                                    ctx_ps=ctx_ps):
                            nc.tensor.matmul(sum_ps[:, off:], ones_sb,
                                             pt[:, off:],
                                             start=(c == 0), stop=(c == nck - 1))
                            nc.tensor.matmul(ctx_ps[:, off:],
                                             v_sb[:, c, h * 128:(h + 1) * 128],
                                             pt[:, off:], start=(c == 0),
                                             stop=(c == nck - 1))

                        pending = []
                        for c in range(nck):
                            ksl = slice(c * 128, (c + 1) * 128)
                            diag = (c // 4 == s)
                            off = 128 * (c % 4) if diag else 0
                            qs2 = slice(s * 512 + off, (s + 1) * 512)
                            sc = psSc.tile([128, 512], dt.float32, tag="sc")
                            nc.tensor.matmul(sc[:, off:], kcT_sb[:, h, ksl],
                                             qcT_sb[:, h, qs2],
                                             start=True, stop=False)
                            nc.tensor.matmul(
                                sc[:, off:],
                                krT_sb[base:base + 64, ksl],
                                qrT_sb[base:base + 64, hp, qs2],
                                start=False, stop=True)
                            pt = ptpool.tile([128, 512], dt.bfloat16, tag="pt")
                            nc.scalar.activation(pt[:, off:], sc[:, off:],
                                                 mybir.ActivationFunctionType.Exp,
                                                 scale=SCALE)
                            if diag:
                                nc.vector.tensor_tensor(
                                    pt[:, off:off + 128], pt[:, off:off + 128],
                                    mask_sb[:, 0, 0:128], mybir.AluOpType.mult)
                            # software pipeline (lag 2): sum/PV trail the
                            # scores by two iterations so the PE rides over
                            # the exp latency of each block
                            pending.append((pt, c, off))
                            if len(pending) > 2:
                                emit_pv(*pending.pop(0))
                        for pp in pending:
                            emit_pv(*pp)
                        # normalize: broadcast the sums, then a fast
                        # approximate reciprocal on all 128 partitions
                        ssb = rpool.tile([1, 512], dt.float32, tag="ssb")
                        nc.scalar.copy(ssb, sum_ps)
                        rb = rpool.tile([128, 512], dt.float32, tag="rb")
                        nc.gpsimd.partition_broadcast(rb, ssb)
                        rb2 = rpool.tile([128, 512], dt.float32, tag="rb2")
                        nc.vector.reciprocal_approx_fast(rb2, rb)
                        nc.vector.tensor_tensor(ctxTn_sb[:, h, qsl], ctx_ps,
                                                rb2, mybir.AluOpType.mult)

                # final WO block (query block TB-1)
                emit_wo_block(TB - 1)

    nc.compile()
    return nc


def _host_tables(l_tokens):
    inv_freq = (1.0 / (10000.0 ** (np.arange(0, RD, 2, dtype=np.float32) / RD))
                ).astype(np.float32)
    pos = np.arange(l_tokens, dtype=np.float32)
    freqs = np.outer(pos, inv_freq).astype(np.float32)
    cos_t = np.concatenate([np.cos(freqs), np.cos(freqs)], axis=-1)
    ss_t = np.concatenate([-np.sin(freqs), np.sin(freqs)], axis=-1)
    return cos_t.astype(np.float32), ss_t.astype(np.float32)


def _host_masks():
    r = np.arange(128)[:, None]
    j = np.arange(512)[None, :]
    m = np.stack([(j >= 128 * mm + r) for mm in range(4)]).astype(np.float32)
    return m.astype(BF)


def make_in_maps(inputs, l_tokens=L):
    """Build the 8 per-core input maps from the full (unsharded) inputs."""
    x = np.asarray(inputs["x"], np.float32)
    cos_t, ss_t = _host_tables(l_tokens)
    maskm = _host_masks()
    ident = np.eye(128, dtype=np.float32).astype(BF)

    xTs = [np.ascontiguousarray(x[b, :l_tokens].T).astype(BF)
           for b in range(x.shape[0])]
    n_blk = l_tokens // 512
    wqd = np.asarray(inputs["wq_down"], np.float32).astype(BF)
    wkvd = np.asarray(inputs["wkv_down"], np.float32).astype(BF)
    wkr = np.asarray(inputs["wk_rope"], np.float32).astype(BF)
    wqu = np.asarray(inputs["wq_up"], np.float32).astype(BF)
    wqr = np.asarray(inputs["wq_rope"], np.float32).astype(BF)
    wku = np.asarray(inputs["wk_up"], np.float32).astype(BF)
    wvu = np.asarray(inputs["wv_up"], np.float32).astype(BF)
    wo = np.asarray(inputs["wo"], np.float32).astype(BF)

    in_maps = []
    for core in range(N_CORES):
        b, g = divmod(core, 4)
        blk = g % n_blk
        p = g % 2
        if n_blk > 1:
            xq2 = np.concatenate(
                [xTs[b][:, p * 512:(p + 1) * 512],
                 xTs[b][:, (p + 2) * 512:(p + 3) * 512]], axis=1)
        else:
            xq2 = xTs[b][:, 0:512]
        in_maps.append({
            "xq": np.ascontiguousarray(xTs[b][:, blk * 512:(blk + 1) * 512]),
            "xq2": np.ascontiguousarray(xq2),
            "cosq": np.ascontiguousarray(cos_t[blk * 512:(blk + 1) * 512]),
            "ssq": np.ascontiguousarray(ss_t[blk * 512:(blk + 1) * 512]),
            "wqd": wqd,
            "wkvd": wkvd,
            "wkr": wkr,
            "wqu": np.ascontiguousarray(wqu[:, g * 512:(g + 1) * 512]),
            "wqr": np.ascontiguousarray(wqr[:, g * 256:(g + 1) * 256]),
            "wku": np.ascontiguousarray(wku[:, g * 512:(g + 1) * 512]),
            "wvu": np.ascontiguousarray(wvu[:, g * 512:(g + 1) * 512]),
            "wo": np.ascontiguousarray(wo[g * 512:(g + 1) * 512, :]),
            "cosT": cos_t,
            "ssT": ss_t,
            "maskm": maskm,
            "ident": ident,
        })
    return in_maps


def kernel(**inputs):
    from concourse.bass_utils import run_bass_kernel_spmd

    if L not in _NC_CACHE:
        _NC_CACHE[L] = build_nc(L)
    nc = _NC_CACHE[L]
    in_maps = make_in_maps(inputs, L)
    res = run_bass_kernel_spmd(nc, in_maps, list(range(N_CORES)))
    out = np.zeros((B, L, H), np.float32)
    for core in range(N_CORES):
        b, _g = divmod(core, 4)
        out[b] += res.results[core]["out"].astype(np.float32)
    return out


# revision 20
# speedup vs baseline: 1.4906x; 1.1703x over previous
"""MLA (multi-head latent attention) Bass kernel for Trainium2, 8 NeuronCores.

Sharding: batch (2) x head-group (4 groups of 4 heads) = 8 cores.
Each core, for its batch b and head group g (pair position p = g % 2):
  - computes kv latent + roped k_rope for its own 512-token block, then ONE
    intra-batch AllGather (cores 0-3 / 4-7) assembles the full-L kv latents
  - computes q latents for token blocks {p, p+2}; two pairwise AllGathers
    (cores {0,1},{2,3},...) assemble blocks {0,1} and {2,3} -- the gathered
    buffers hold blocks in a core-independent order, keeping the program
    SPMD-uniform
  - up-projects q/k/v for its 4 heads, runs causal attention, and a partial
    output  out_partial = ctx_g @ wo[512g:512(g+1), :].
Host sums the 4 partial outputs per batch (the wo row-shard reduction).

All host-supplied tensors are pre-tiled to SBUF layout (partition-major)
so every weight/activation load is a single contiguous-per-partition DMA
(~128 descriptors); descriptor generation on the sequencers stays off the
critical path.  Gather buffers use the same partition-major layout.

Softmax row-sums are computed with one ones-matmul per 4-chunk quad (the
exp tiles are pre-accumulated pairwise on the Vector engine), which keeps
the PE free for score/PV work.

All matmuls run in bf16 with fp32 PSUM accumulation. Softmax skips the
row-max subtraction (scores are O(+-10), exp stays in fp32 range).
"""
import math
import sys

sys.path.insert(0, "/opt/trn_rl_repo")

import numpy as np
import ml_dtypes

B, L, H = 2, 2048, 2048
NH, HD, RD = 16, 128, 64
QR, KVR = 768, 512
NHC = 4            # heads per core
N_CORES = 8
SCALE = 1.0 / math.sqrt(HD + RD)
BF = ml_dtypes.bfloat16

_NC_CACHE = {}


def build_nc(l_tokens=L):
    import concourse.bass as bass  # noqa: F401
    import concourse.tile as tile
    from concourse import bacc, mybir

    dt = mybir.dt
    Lk = l_tokens
    assert Lk % 512 == 0
    TB = Lk // 512          # 512-token blocks
    assert TB in (1, 4)
    KC = Lk // 128          # 128-token chunks
    HB = H // 512           # output column blocks
    NQ, NKV = QR // 128, KVR // 128
    HK = H // 128
    NQB = 1 if TB == 1 else 2   # own q-latent blocks per core

    nc = bacc.Bacc("TRN2", target_bir_lowering=False, debug=False,
                   num_devices=N_CORES)

    def din(name, shape, d=dt.bfloat16):
        return nc.dram_tensor(name, shape, d, kind="ExternalInput").ap()

    # all inputs pre-tiled to partition-major SBUF layout by the host
    xq = din("xq", [128, HK, 512])          # own 512-token block of x^T
    xq2 = din("xq2", [NQB, 128, HK, 512])   # q-latent blocks {p, p+2}
    wqd = din("wqd", [128, NQ, HK, 128])
    wkvd = din("wkvd", [NKV, 128, HK, 128])
    wkr = din("wkr", [128, HK, RD])
    wqu = din("wqu", [128, NQ, NHC * HD])
    wqr = din("wqr", [128, NQ, NHC * RD])
    wku = din("wku", [128, NKV, NHC * HD])
    wvu = din("wvu", [128, NKV, NHC * HD])
    wo = din("wo", [128, NHC, H])
    cosT = din("cosT", [128, KC, RD], dt.float32)
    ssT = din("ssT", [128, KC, RD], dt.float32)   # [-sin | +sin]
    cosq = din("cosq", [128, 4, RD], dt.float32)  # own-block slices
    ssq = din("ssq", [128, 4, RD], dt.float32)
    maskm = din("maskm", [128, 4, 512])           # multiplicative causal masks
    ident = din("ident", [128, 128])
    out = nc.dram_tensor("out", [Lk, H], dt.bfloat16, kind="ExternalOutput").ap()

    # gather buffers, partition-major: latent columns + a krope-T region
    CCG = [[0, 1, 2, 3], [4, 5, 6, 7]]
    CCP = [[0, 1], [2, 3], [4, 5], [6, 7]]
    skv_d = nc.dram_tensor("skv_d", [128, NKV + 1, 512], dt.bfloat16).ap()
    sql_a = nc.dram_tensor("sql_a", [128, NQ, 512], dt.bfloat16).ap()
    if TB > 1:
        gkv_d = nc.dram_tensor("gkv_d", [TB * 128, NKV + 1, 512],
                               dt.bfloat16).ap()
        sql_b = nc.dram_tensor("sql_b", [128, NQ, 512], dt.bfloat16).ap()
        gql_a = nc.dram_tensor("gql_a", [2 * 128, NQ, 512], dt.bfloat16).ap()
        gql_b = nc.dram_tensor("gql_b", [2 * 128, NQ, 512], dt.bfloat16).ap()
    else:
        gkv_d = skv_d
        sql_b = gql_a = gql_b = sql_a

    with tile.TileContext(nc) as tc:
        with (
            tc.tile_pool(name="const", bufs=1) as cpool,
            tc.tile_pool(name="attn", bufs=1) as apool,
        ):
            # ---- whole-program constants ----
            wkr_sb = cpool.tile([128, HK, RD], dt.bfloat16, name="wkr_sb")
            cosq_sb = cpool.tile([128, 4, RD], dt.float32, name="cosq_sb")
            ssq_sb = cpool.tile([128, 4, RD], dt.float32, name="ssq_sb")
            id_sb = cpool.tile([128, 128], dt.bfloat16, name="id_sb")
            cos_sb = cpool.tile([128, KC, RD], dt.float32, name="cos_sb")
            ss_sb = cpool.tile([128, KC, RD], dt.float32, name="ss_sb")

            # ---- persistent attention operands ----
            qcT_sb = apool.tile([128, NHC, Lk], dt.bfloat16, name="qcT_sb")
            kcT_sb = apool.tile([128, NHC, Lk], dt.bfloat16, name="kcT_sb")
            qrT_sb = apool.tile([128, 2, Lk], dt.bfloat16, name="qrT_sb")
            krT_sb = apool.tile([128, Lk], dt.bfloat16, name="krT_sb")
            v_sb = apool.tile([128, KC, 512], dt.bfloat16, name="v_sb")

            # ================= Phase 1: projections =================
            with (
                tc.tile_pool(name="p1w", bufs=1) as wpool,
                tc.tile_pool(name="wcolp", bufs=2) as wcpool,
                tc.tile_pool(name="xbp", bufs=2) as xpool,
                tc.tile_pool(name="qlp", bufs=2) as qlpool,
                tc.tile_pool(name="kvbp", bufs=3) as kvbpool,
                tc.tile_pool(name="p1s", bufs=2) as spool,
                tc.tile_pool(name="psA", bufs=2, space="PSUM") as psA,
                tc.tile_pool(name="psSmall", bufs=2, space="PSUM") as psS,
                tc.tile_pool(name="psTp", bufs=2, space="PSUM") as psT,
            ):
                # --- 1a: own-block kv latent + roped k_rope ---
                xb0 = xpool.tile([128, HK, 512], dt.bfloat16, tag="xb")
                xqb0 = xpool.tile([128, HK, 512], dt.bfloat16, tag="xb")
                for m in range(NKV):
                    wc = wcpool.tile([128, HK, 128], dt.bfloat16, tag="wcol")
                    nc.sync.dma_start(wc, wkvd[m])
                    if m == 0:
                        nc.sync.dma_start(xb0, xq)
                        nc.sync.dma_start(wkr_sb, wkr)
                        nc.sync.dma_start(cosq_sb, cosq)
                        nc.sync.dma_start(ssq_sb, ssq)
                        nc.sync.dma_start(id_sb, ident)
                        # q-side inputs, in order of first use
                        nc.sync.dma_start(xqb0, xq2[0])
                        wqd_sb = wpool.tile([128, NQ, HK, 128], dt.bfloat16,
                                            name="wqd_sb")
                        nc.sync.dma_start(wqd_sb, wqd)
                        wku_sb = wpool.tile([128, NKV, 512], dt.bfloat16,
                                            name="wku_sb")
                        nc.sync.dma_start(wku_sb, wku)
                        wvu_sb = wpool.tile([128, NKV, 512], dt.bfloat16,
                                            name="wvu_sb")
                        nc.sync.dma_start(wvu_sb, wvu)
                        wqu_sb = wpool.tile([128, NQ, 512], dt.bfloat16,
                                            name="wqu_sb")
                        nc.sync.dma_start(wqu_sb, wqu)
                        wqr_sb = wpool.tile([128, NQ, 256], dt.bfloat16,
                                            name="wqr_sb")
                        nc.sync.dma_start(wqr_sb, wqr)
                        nc.sync.dma_start(cos_sb, cosT)
                        nc.sync.dma_start(ss_sb, ssT)
                    ps = psA.tile([128, 512], dt.float32, tag="mm")
                    for k in range(HK):
                        nc.tensor.matmul(ps, wc[:, k, :], xb0[:, k, :],
                                         start=(k == 0), stop=(k == HK - 1))
                    lt = spool.tile([128, 512], dt.bfloat16, tag="lat")
                    nc.scalar.copy(lt, ps)
                    nc.sync.dma_start(skv_d[:, m, :], lt)

                # roped k_rope for own block (feature-on-partition, 64 rows)
                for tc2 in range(4):
                    tsl = slice(tc2 * 128, (tc2 + 1) * 128)
                    kr_ps = psS.tile([128, RD], dt.float32, tag="sm")
                    for k in range(HK):
                        nc.tensor.matmul(kr_ps, xb0[:, k, tsl], wkr_sb[:, k, :],
                                         start=(k == 0), stop=(k == HK - 1))
                    t1 = spool.tile([128, RD], dt.float32, tag="t1")
                    nc.vector.tensor_tensor(t1, kr_ps, cosq_sb[:, tc2, :],
                                            mybir.AluOpType.mult)
                    t2 = spool.tile([128, RD], dt.float32, tag="t2")
                    nc.vector.tensor_tensor(t2[:, 0:32], kr_ps[:, 32:64],
                                            ssq_sb[:, tc2, 0:32],
                                            mybir.AluOpType.mult)
                    nc.vector.tensor_tensor(t2[:, 32:64], kr_ps[:, 0:32],
                                            ssq_sb[:, tc2, 32:64],
                                            mybir.AluOpType.mult)
                    krb = spool.tile([128, RD], dt.bfloat16, tag="krb")
                    nc.vector.tensor_tensor(krb, t1, t2, mybir.AluOpType.add)
                    ktp = psT.tile([128, 128], dt.bfloat16, tag="tp")
                    nc.tensor.transpose(ktp[0:RD, :], krb, id_sb)
                    kt = spool.tile([RD, 128], dt.bfloat16, tag="krt")
                    nc.vector.tensor_copy(kt, ktp[0:RD, :])
                    nc.sync.dma_start(skv_d[0:RD, NKV, tsl], kt)

                # second own q block: recycles xb0's buffer (all of whose
                # readers were emitted above)
                if NQB > 1:
                    xqb1 = xpool.tile([128, HK, 512], dt.bfloat16, tag="xb")
                    nc.sync.dma_start(xqb1, xq2[1])
                else:
                    xqb1 = xqb0

                # --- CC1: gather kv latents across the 4-core batch group ---
                if TB > 1:
                    nc.gpsimd.collective_compute(
                        "AllGather", mybir.AluOpType.bypass,
                        replica_groups=CCG, ins=[skv_d], outs=[gkv_d])
                kvb0 = kvbpool.tile([128, NKV, 512], dt.bfloat16, tag="kvb")
                kvbs = [kvb0]
                nc.sync.dma_start(kvb0, gkv_d[0:128, 0:NKV, :])

                # --- 1c: q latents for own blocks {p, p+2} ---
                for j in range(NQB):
                    xjb = xqb0 if j == 0 else xqb1
                    sql = sql_a if j == 0 else sql_b
                    for m in range(NQ):
                        ps = psA.tile([128, 512], dt.float32, tag="mm")
                        for k in range(HK):
                            nc.tensor.matmul(ps, wqd_sb[:, m, k, :],
                                             xjb[:, k, :],
                                             start=(k == 0), stop=(k == HK - 1))
                        lt = spool.tile([128, 512], dt.bfloat16, tag="lat")
                        nc.scalar.copy(lt, ps)
                        nc.sync.dma_start(sql[:, m, :], lt)
                    if TB > 1:
                        nc.gpsimd.collective_compute(
                            "AllGather", mybir.AluOpType.bypass,
                            replica_groups=CCP, ins=[sql],
                            outs=[gql_a if j == 0 else gql_b])
                    if j == 0 and TB > 1:
                        kvb1 = kvbpool.tile([128, NKV, 512], dt.bfloat16,
                                            tag="kvb")
                        kvbs.append(kvb1)
                        nc.sync.dma_start(kvb1, gkv_d[128:256, 0:NKV, :])

                if TB > 1:
                    kvb2 = kvbpool.tile([128, NKV, 512], dt.bfloat16,
                                        tag="kvb")
                    kvbs.append(kvb2)
                    nc.sync.dma_start(kvb2, gkv_d[256:384, 0:NKV, :])
                for tb in range(TB):
                    ts0 = tb * 512
                    nc.sync.dma_start(krT_sb[0:RD, ts0:ts0 + 512],
                                      gkv_d[tb * 128:tb * 128 + RD, NKV, :])
                    nc.sync.dma_start(krT_sb[RD:2 * RD, ts0:ts0 + 512],
                                      gkv_d[tb * 128:tb * 128 + RD, NKV, :])

                # --- 1d: kcT + v from the gathered kv latents ---
                for tb in range(TB):
                    ts0 = tb * 512
                    kvb = kvbs[tb]
                    for hc in range(NHC):
                        ps = psA.tile([128, 512], dt.float32, tag="mm")
                        for m in range(NKV):
                            nc.tensor.matmul(ps,
                                             wku_sb[:, m, hc * 128:(hc + 1) * 128],
                                             kvb[:, m, :],
                                             start=(m == 0), stop=(m == NKV - 1))
                        nc.scalar.copy(kcT_sb[:, hc, ts0:ts0 + 512], ps)
                    for tc2 in range(4):
                        ps = psA.tile([128, 512], dt.float32, tag="mm")
                        for m in range(NKV):
                            nc.tensor.matmul(ps,
                                             kvb[:, m, tc2 * 128:(tc2 + 1) * 128],
                                             wvu_sb[:, m, :],
                                             start=(m == 0), stop=(m == NKV - 1))
                        nc.scalar.copy(v_sb[:, tb * 4 + tc2, :], ps)
                    if tb == 0 and TB > 1:
                        # block-3 kv: safe to recycle kvb0's buffer now that
                        # block-0's readers are all emitted
                        kvb3 = kvbpool.tile([128, NKV, 512], dt.bfloat16,
                                            tag="kvb")
                        kvbs.append(kvb3)
                        nc.sync.dma_start(kvb3, gkv_d[384:512, 0:NKV, :])

                # --- 1e: q up-projections from the gathered q latents ---
                for r in range(TB):
                    ts0 = r * 512
                    qlb = qlpool.tile([128, NQ, 512], dt.bfloat16, tag="qlb")
                    gq = gql_a if r < 2 else gql_b
                    row0 = (r % 2) * 128
                    nc.sync.dma_start(qlb, gq[row0:row0 + 128, :, :])

                    for hc in range(NHC):
                        ps = psA.tile([128, 512], dt.float32, tag="mm")
                        for m in range(NQ):
                            nc.tensor.matmul(ps,
                                             wqu_sb[:, m, hc * 128:(hc + 1) * 128],
                                             qlb[:, m, :],
                                             start=(m == 0), stop=(m == NQ - 1))
                        nc.scalar.copy(qcT_sb[:, hc, ts0:ts0 + 512], ps)

                    for tc2 in range(4):
                        gc = r * 4 + tc2
                        tsl = slice(tc2 * 128, (tc2 + 1) * 128)
                        qr_ps = psS.tile([128, NHC * RD], dt.float32, tag="sm")
                        for m in range(NQ):
                            nc.tensor.matmul(qr_ps, qlb[:, m, tsl],
                                             wqr_sb[:, m, :],
                                             start=(m == 0), stop=(m == NQ - 1))
                        qrv = qr_ps.rearrange("p (h d) -> p h d", d=RD)
                        q1 = spool.tile([128, NHC, RD], dt.float32, tag="q1")
                        nc.vector.tensor_tensor(
                            q1, qrv,
                            cos_sb[:, gc, None, :].to_broadcast([128, NHC, RD]),
                            mybir.AluOpType.mult)
                        q2 = spool.tile([128, NHC, RD], dt.float32, tag="q2")
                        nc.vector.tensor_tensor(
                            q2[:, :, 0:32], qrv[:, :, 32:64],
                            ss_sb[:, gc, None, 0:32].to_broadcast([128, NHC, 32]),
                            mybir.AluOpType.mult)
                        nc.vector.tensor_tensor(
                            q2[:, :, 32:64], qrv[:, :, 0:32],
                            ss_sb[:, gc, None, 32:64].to_broadcast([128, NHC, 32]),
                            mybir.AluOpType.mult)
                        qrb = spool.tile([128, NHC * RD], dt.bfloat16, tag="qrb")
                        nc.vector.tensor_tensor(
                            qrb.rearrange("p (h d) -> p h d", d=RD), q1, q2,
                            mybir.AluOpType.add)
                        for hp in range(2):
                            qtp = psT.tile([128, 128], dt.bfloat16, tag="tp")
                            nc.tensor.transpose(
                                qtp, qrb[:, hp * 128:(hp + 1) * 128], id_sb)
                            nc.vector.tensor_copy(
                                qrT_sb[:, hp, gc * 128:(gc + 1) * 128], qtp)

            # ============ Phase 2+3: attention, WO interleaved ============
            # WO for query-block s is emitted right after attention block s,
            # so its matmuls fill the next block's exp-latency PE gaps
            # instead of piling up into a copy-chain-bound tail.
            with (
                tc.tile_pool(name="c2", bufs=1) as c2pool,
                tc.tile_pool(name="ptp", bufs=6) as ptpool,
                tc.tile_pool(name="accp", bufs=2) as accpool,
                tc.tile_pool(name="obp", bufs=4) as opool,
                tc.tile_pool(name="recp", bufs=2) as rpool,
                tc.tile_pool(name="psSc", bufs=3, space="PSUM") as psSc,
                tc.tile_pool(name="psCtx", bufs=2, space="PSUM") as psC,
                tc.tile_pool(name="psSum", bufs=1, space="PSUM") as psM,
                tc.tile_pool(name="psWo", bufs=2, space="PSUM") as psW,
            ):
                ctxTn_sb = c2pool.tile([128, NHC, Lk], dt.bfloat16,
                                       name="ctxTn_sb")
                mask_sb = c2pool.tile([128, 4, 512], dt.bfloat16,
                                      name="mask_sb")
                nc.sync.dma_start(mask_sb, maskm)
                ones_sb = c2pool.tile([128, 1], dt.bfloat16, name="ones_sb")
                nc.vector.memset(ones_sb, 1.0)
                wo_sb = c2pool.tile([128, NHC, H], dt.bfloat16, name="wo_sb")
                nc.sync.dma_start(wo_sb, wo)

                def emit_wo_block(s):
                    for tc3 in range(4 * s, 4 * s + 4):
                        csl = slice(tc3 * 128, (tc3 + 1) * 128)
                        for nb in range(HB):
                            nsl = slice(nb * 512, (nb + 1) * 512)
                            po = psW.tile([128, 512], dt.float32, tag="wo")
                            for h in range(NHC):
                                nc.tensor.matmul(po, ctxTn_sb[:, h, csl],
                                                 wo_sb[:, h, nsl],
                                                 start=(h == 0),
                                                 stop=(h == NHC - 1))
                            ob = opool.tile([128, 512], dt.bfloat16, tag="ob")
                            nc.vector.tensor_copy(ob, po)
                            nc.sync.dma_start(out[csl, nsl], ob)

                for s in range(TB):
                    qsl = slice(s * 512, (s + 1) * 512)
                    if s > 0:
                        emit_wo_block(s - 1)
                    for h in range(NHC):
                        hp, half = divmod(h, 2)
                        base = 64 * half
                        nck = 4 * s + 4
                        nquads = s + 1
                        ctx_ps = psC.tile([128, 512], dt.float32, tag="ctx")
                        sum_ps = psM.tile([1, 512], dt.float32, tag="sum")

                        def emit_pv(pt, c, off, nck=nck, h=h, ctx_ps=ctx_ps):
                            nc.tensor.matmul(ctx_ps[:, off:],
                                             v_sb[:, c, h * 128:(h + 1) * 128],
                                             pt[:, off:], start=(c == 0),
                                             stop=(c == nck - 1))

                        pending = []
                        acc = None
                        accfirst = None
                        for c in range(nck):
                            ksl = slice(c * 128, (c + 1) * 128)
                            diag = (c // 4 == s)
                            off = 128 * (c % 4) if diag else 0
                            qs2 = slice(s * 512 + off, (s + 1) * 512)
                            sc = psSc.tile([128, 512], dt.float32, tag="sc")
                            nc.tensor.matmul(sc[:, off:], kcT_sb[:, h, ksl],
                                             qcT_sb[:, h, qs2],
                                             start=True, stop=False)
                            nc.tensor.matmul(
                                sc[:, off:],
                                krT_sb[base:base + 64, ksl],
                                qrT_sb[base:base + 64, hp, qs2],
                                start=False, stop=True)
                            pt = ptpool.tile([128, 512], dt.bfloat16, tag="pt")
                            nc.scalar.activation(pt[:, off:], sc[:, off:],
                                                 mybir.ActivationFunctionType.Exp,
                                                 scale=SCALE)
                            if diag:
                                nc.vector.tensor_tensor(
                                    pt[:, off:off + 128], pt[:, off:off + 128],
                                    mask_sb[:, 0, 0:128], mybir.AluOpType.mult)
                            # quad-accumulate the exp tiles on Vector; one
                            # ones-matmul per quad computes the row sums
                            qpos = c % 4
                            if qpos == 0:
                                acc = accpool.tile([128, 512], dt.bfloat16,
                                                   tag="acc")
                                accfirst = pt
                            elif qpos == 1:
                                if off:
                                    nc.vector.tensor_copy(acc, accfirst)
                                    nc.vector.tensor_tensor(
                                        acc[:, off:], acc[:, off:], pt[:, off:],
                                        mybir.AluOpType.add)
                                else:
                                    nc.vector.tensor_tensor(
                                        acc, accfirst, pt, mybir.AluOpType.add)
                            else:
                                if off:
                                    nc.vector.tensor_tensor(
                                        acc[:, off:], acc[:, off:], pt[:, off:],
                                        mybir.AluOpType.add)
                                else:
                                    nc.vector.tensor_tensor(
                                        acc, acc, pt, mybir.AluOpType.add)
                            if qpos == 3:
                                quad = c // 4
                                nc.tensor.matmul(sum_ps, ones_sb, acc,
                                                 start=(quad == 0),
                                                 stop=(quad == nquads - 1))
                            # software pipeline (lag 2): PV trails the scores
                            # by two iterations so the PE rides over the exp
                            # latency of each block
                            pending.append((pt, c, off))
                            if len(pending) > 2:
                                emit_pv(*pending.pop(0))
                        for pp in pending:
                            emit_pv(*pp)
                        # normalize: broadcast the sums, then a fast
                        # approximate reciprocal on all 128 partitions
                        ssb = rpool.tile([1, 512], dt.float32, tag="ssb")
                        nc.scalar.copy(ssb, sum_ps)
                        rb = rpool.tile([128, 512], dt.float32, tag="rb")
                        nc.gpsimd.partition_broadcast(rb, ssb)
                        rb2 = rpool.tile([128, 512], dt.float32, tag="rb2")
                        nc.vector.reciprocal_approx_fast(rb2, rb)
                        nc.vector.tensor_tensor(ctxTn_sb[:, h, qsl], ctx_ps,
                                                rb2, mybir.AluOpType.mult)

                # final WO block (query block TB-1)
                emit_wo_block(TB - 1)

    nc.compile()
    return nc


def _host_tables(l_tokens):
    inv_freq = (1.0 / (10000.0 ** (np.arange(0, RD, 2, dtype=np.float32) / RD))
                ).astype(np.float32)
    pos = np.arange(l_tokens, dtype=np.float32)
    freqs = np.outer(pos, inv_freq).astype(np.float32)
    cos_t = np.concatenate([np.cos(freqs), np.cos(freqs)], axis=-1)
    ss_t = np.concatenate([-np.sin(freqs), np.sin(freqs)], axis=-1)
    return cos_t.astype(np.float32), ss_t.astype(np.float32)


def _host_masks():
    r = np.arange(128)[:, None]
    j = np.arange(512)[None, :]
    m = np.stack([(j >= 128 * mm + r) for mm in range(4)]).astype(np.float32)
    return m.astype(BF)


def _ptile(a, p=128):
    """[K*p, N...] -> [p, K, N...] partition-major, contiguous."""
    a = np.ascontiguousarray(a)
    k = a.shape[0] // p
    return np.ascontiguousarray(
        a.reshape((k, p) + a.shape[1:]).swapaxes(0, 1))


def make_in_maps(inputs, l_tokens=L):
    """Build the 8 per-core input maps from the full (unsharded) inputs."""
    x = np.asarray(inputs["x"], np.float32)
    cos_t, ss_t = _host_tables(l_tokens)
    maskm = np.ascontiguousarray(_host_masks().swapaxes(0, 1))  # [128,4,512]
    ident = np.eye(128, dtype=np.float32).astype(BF)

    xTs = [np.ascontiguousarray(x[b, :l_tokens].T).astype(BF)
           for b in range(x.shape[0])]
    n_blk = l_tokens // 512
    NQ, NKV = QR // 128, KVR // 128
    wqd = np.asarray(inputs["wq_down"], np.float32).astype(BF)
    wkvd = np.asarray(inputs["wkv_down"], np.float32).astype(BF)
    wkr = np.asarray(inputs["wk_rope"], np.float32).astype(BF)
    wqu = np.asarray(inputs["wq_up"], np.float32).astype(BF)
    wqr = np.asarray(inputs["wq_rope"], np.float32).astype(BF)
    wku = np.asarray(inputs["wk_up"], np.float32).astype(BF)
    wvu = np.asarray(inputs["wv_up"], np.float32).astype(BF)
    wo = np.asarray(inputs["wo"], np.float32).astype(BF)

    # wqd: [p, m, k, 128];  wkvd: [m, p, k, 128]
    wqd_t = np.ascontiguousarray(
        wqd.reshape(H // 128, 128, NQ, 128).transpose(1, 2, 0, 3))
    wkvd_t = np.ascontiguousarray(
        wkvd.reshape(H // 128, 128, NKV, 128).transpose(2, 1, 0, 3))
    wkr_t = _ptile(wkr)
    cos_tt = _ptile(cos_t)
    ss_tt = _ptile(ss_t)
    wqu_t = _ptile(wqu)
    wqr_t = _ptile(wqr)
    wku_t = _ptile(wku)
    wvu_t = _ptile(wvu)

    in_maps = []
    for core in range(N_CORES):
        b, g = divmod(core, 4)
        blk = g % n_blk
        p = g % 2
        if n_blk > 1:
            xq2 = np.stack(
                [_ptile(np.ascontiguousarray(
                    xTs[b][:, blk2 * 512:(blk2 + 1) * 512]))
                 for blk2 in (p, p + 2)])
        else:
            xq2 = _ptile(xTs[b][:, 0:512])[None]
        in_maps.append({
            "xq": _ptile(np.ascontiguousarray(
                xTs[b][:, blk * 512:(blk + 1) * 512])),
            "xq2": np.ascontiguousarray(xq2),
            "cosq": _ptile(np.ascontiguousarray(
                cos_t[blk * 512:(blk + 1) * 512])),
            "ssq": _ptile(np.ascontiguousarray(
                ss_t[blk * 512:(blk + 1) * 512])),
            "wqd": wqd_t,
            "wkvd": wkvd_t,
            "wkr": wkr_t,
            "wqu": np.ascontiguousarray(wqu_t[:, :, g * 512:(g + 1) * 512]),
            "wqr": np.ascontiguousarray(wqr_t[:, :, g * 256:(g + 1) * 256]),
            "wku": np.ascontiguousarray(wku_t[:, :, g * 512:(g + 1) * 512]),
            "wvu": np.ascontiguousarray(wvu_t[:, :, g * 512:(g + 1) * 512]),
            "wo": _ptile(np.ascontiguousarray(wo[g * 512:(g + 1) * 512, :])),
            "cosT": cos_tt,
            "ssT": ss_tt,
            "maskm": maskm,
            "ident": ident,
        })
    return in_maps


def kernel(**inputs):
    from concourse.bass_utils import run_bass_kernel_spmd

    if L not in _NC_CACHE:
        _NC_CACHE[L] = build_nc(L)
    nc = _NC_CACHE[L]
    in_maps = make_in_maps(inputs, L)
    res = run_bass_kernel_spmd(nc, in_maps, list(range(N_CORES)))
    out = np.zeros((B, L, H), np.float32)
    for core in range(N_CORES):
        b, _g = divmod(core, 4)
        out[b] += res.results[core]["out"].astype(np.float32)
    return out
